# revision 2
# baseline (speedup 1.0000x reference)
"""DILATE loss (soft-DTW fwd + grad, gamma=0.01 ~ hard-min) on TRN2.

Single-core, two sequential passes of 32 samples each (the DP instructions
span all 128 SBUF partitions: quadrant q holds col-block q for 32 samples on
lanes 32q+0..31). Per pass, the skewed wavefront runs 260 slots; slot t of
block q holds DP row i = t - q in a 65-float record [chain | 64 cols];
tensor_tensor_scan computes each row's min-plus recurrence in one
instruction. The soft-DTW gradient is the hard argmin-mask linear recurrence
run as a reversed scan; masks are equality-derived in batched chunks and
bounced through DRAM.

Wall-clock is dominated by the axon tunnel round trip (~85ms per
host-initiated op), so the host interface is minimized: one cached jax.jit
executable, one [128,256] f32 input (raw y values; the skewed dT/dO layouts
are built on-device by DMA), one [128,4] f32 output (pt/ps packed per pass),
and the index-difference omega matrix baked into the NEFF as a constant.
"""
import numpy as np
import ml_dtypes

bf16 = ml_dtypes.bfloat16
f32 = np.float32

ALPHA = 0.5
BIG = 1e8
B, N = 64, 256
Q, C = 4, 65
S, SE = 260, 262
NREPS = 2          # passes per core; 32 samples each
SPP = 32           # samples per pass
MCH = 8            # mask-phase chunk (slots)
WCH = 8            # backward mask window stride (slots); window covers WCH+1 slots

_cache = {}


def _mx_const():
    """[128, S*C] bf16: mx[32q+s, t, 1+l] = (i - j) for i=t-q, j=q*64+1+l,
    zeroed outside 1<=i<=N and in the chain col. Input-independent."""
    mx = np.zeros((128, S, C), f32)
    t = np.arange(S)
    for q in range(Q):
        i = t - q
        valid = (i >= 1) & (i <= N)
        j = q * 64 + np.arange(1, 65)
        X = (i[:, None] - j[None, :]).astype(f32)
        X[~valid, :] = 0.0
        mx[32 * q:32 * (q + 1), :, 1:] = X[None, :, :]
    return mx.reshape(128, S * C).astype(bf16)


def _build(nreps=NREPS, phases="dfmbx", fwdops="cms"):
    import concourse.bacc as bacc
    import concourse.tile as tile
    import concourse.mybir as mybir
    from contextlib import ExitStack

    dt = mybir.dt
    Alu = mybir.AluOpType

    nc = bacc.Bacc("TRN2", target_bir_lowering=False, debug=False)
    # rows 64r+0..31: y_true samples 32r..32r+31; rows 64r+32..63: y_pred
    Y_d = nc.dram_tensor("Y", [128, N], dt.float32, kind="ExternalInput").ap()
    out_d = nc.dram_tensor("out", [128, 2 * nreps], dt.float32,
                           kind="ExternalOutput").ap()
    mx_d = nc.inline_tensor(_mx_const(), name="mx").ap()
    mU_d = nc.dram_tensor("mU_s", [128, SE * C], dt.bfloat16).ap()
    mD_d = nc.dram_tensor("mD_s", [128, SE * C], dt.bfloat16).ap()
    mL_d = nc.dram_tensor("mL_s", [128, SE * C], dt.bfloat16).ap()

    with tile.TileContext(nc) as tc:
        with ExitStack() as ctx:
            big = ctx.enter_context(tc.tile_pool(name="big", bufs=1))
            st_pool = ctx.enter_context(tc.tile_pool(name="stage", bufs=2))
            win_pool = ctx.enter_context(tc.tile_pool(name="win", bufs=2))
            sc_pool = ctx.enter_context(tc.tile_pool(name="scr", bufs=2))

            h = big.tile([128, S * C], dt.float32, tag="h")
            d = big.tile([128, S * C], dt.bfloat16, tag="d")
            E = big.tile([128, SE * C], dt.float32, tag="E")
            dT = big.tile([128, S], dt.float32, tag="dT")
            dO = big.tile([128, 64], dt.float32, tag="dO")
            c0 = big.tile([128, C], dt.float32, tag="c0")
            c1 = big.tile([128, C], dt.float32, tag="c1")
            G0 = big.tile([128, 66], dt.float32, tag="G0")
            G1t = big.tile([128, 66], dt.float32, tag="G1t")
            S0 = big.tile([128, 66], dt.float32, tag="S0")
            S1t = big.tile([128, 66], dt.float32, tag="S1t")
            zb = big.tile([128, 2 * C], dt.bfloat16, tag="zb")
            pt_t = big.tile([128, 1], dt.float32, tag="pt_t")
            c_tiles = [c0, c1]
            G_tiles = [G0, G1t]
            S_tiles = [S0, S1t]

            for rep in range(nreps):
                # skewed dT/dO layout straight from DRAM: quadrant q gets
                # y_true shifted by q+1 cols / y_pred col-block q
                nc.vector.memset(dT[:], 0.0)
                for q in range(Q):
                    nc.sync.dma_start(
                        out=dT[32 * q:32 * q + 32, q + 1:q + 257],
                        in_=Y_d[64 * rep:64 * rep + 32, :])
                    nc.sync.dma_start(
                        out=dO[32 * q:32 * q + 32, 0:64],
                        in_=Y_d[64 * rep + 32:64 * rep + 64, 64 * q:64 * q + 64])
                # E zero on gpsimd (runs concurrent with fwd on DVE)
                nc.gpsimd.memset(E[:], 0.0)
                nc.gpsimd.memset(zb[:], 0.0)

                # D build: d[p, t*C+1+jl] = (dT[p,t]-dO[p,jl])^2  (bf16)
                if "d" not in phases:
                    continue
                nc.vector.memset(d[:], 0.0)
                DCH = 33
                for k0 in range(0, S, DCH):
                    k1 = min(k0 + DCH, S)
                    d3 = d[:].rearrange("p (s c) -> p s c", c=C)[:, k0:k1, 1:]
                    nc.vector.tensor_tensor(
                        d3, dT[:, k0:k1].unsqueeze(2).broadcast_to([128, k1 - k0, 64]),
                        dO[:].unsqueeze(1).broadcast_to([128, k1 - k0, 64]), Alu.subtract)
                    nc.vector.tensor_tensor(d3, d3, d3, Alu.mult)

                # fwd prefills
                for q in range(Q):
                    nc.vector.memset(h[32 * q:32 * q + 32, q * C:(q + 1) * C], BIG)
                nc.vector.memset(h[0:32, 0:1], 0.0)
                for ct in c_tiles:
                    nc.vector.memset(ct[0:32, 0:1], BIG)
                for gt in G_tiles:
                    nc.vector.memset(gt[:, 0:1], 0.0)
                    nc.vector.memset(gt[96:128, 65:66], 0.0)

                # ---------------- forward ----------------
                if "f" not in phases:
                    continue
                def prange(qlo, qhi):
                    P0, P1 = 32 * qlo, 32 * qhi + 32
                    cnt = P1 - P0
                    if not (cnt <= 32 or P0 == 0 or (P0 == 64 and cnt <= 64)):
                        P0 = 0
                    return P0, P1

                for t in range(1, S):
                    qlo, qhi = max(0, t - 256), min(3, t - 1)
                    P0, P1 = prange(qlo, qhi)
                    ct = c_tiles[t % 2]
                    if "c" in fwdops:
                        for q in range(max(1, qlo), qhi + 1):
                            nc.gpsimd.tensor_copy(
                                ct[32 * q:32 * q + 32, 0:1],
                                h[32 * (q - 1):32 * q, (t - 1) * C + 64:(t - 1) * C + 65])
                    if "m" in fwdops:
                        nc.vector.tensor_tensor(
                            ct[P0:P1, 1:65],
                            h[P0:P1, (t - 1) * C + 1:(t - 1) * C + 65],
                            h[P0:P1, (t - 1) * C:(t - 1) * C + 64], Alu.min)
                    if "s" in fwdops:
                        # state = min(c'_j, state) + d_j  (c' excludes d; chain in c'[0])
                        nc.vector.tensor_tensor_scan(
                            h[P0:P1, t * C:t * C + 65],
                            ct[P0:P1, 0:65],
                            d[P0:P1, t * C:t * C + 65], float(BIG), Alu.min, Alu.add)

                # loss_shape partials: V[N,M] per sample
                nc.sync.dma_start(
                    out=out_d[96:128, 2 * rep + 1:2 * rep + 2],
                    in_=h[96:128, 259 * C + 64:259 * C + 65])

                # ---------------- mask phase ----------------
                if "m" not in phases:
                    continue
                for s0 in range(1, S, MCH):
                    s1 = min(s0 + MCH, S)
                    ns = s1 - s0
                    cX = sc_pool.tile([128, MCH * C], dt.float32, tag="cX")
                    mu = st_pool.tile([128, MCH * C], dt.bfloat16, tag="mu")
                    md = st_pool.tile([128, MCH * C], dt.bfloat16, tag="md")
                    ml = st_pool.tile([128, MCH * C], dt.bfloat16, tag="ml")
                    hv = h[:].rearrange("p (s c) -> p s c", c=C)
                    dv = d[:].rearrange("p (s c) -> p s c", c=C)
                    cXv = cX[:].rearrange("p (s c) -> p s c", c=C)[:, 0:ns, :]
                    for m_t, hoff in ((mu, hv[:, s0 - 1:s1 - 1, 1:]),
                                      (md, hv[:, s0 - 1:s1 - 1, 0:64]),
                                      (ml, hv[:, s0:s1, 0:64])):
                        nc.vector.tensor_tensor(cXv[:, :, 1:], dv[:, s0:s1, 1:], hoff, Alu.add)
                        mv = m_t[:].rearrange("p (s c) -> p s c", c=C)[:, 0:ns, :]
                        nc.vector.tensor_tensor(mv[:, :, 1:], hv[:, s0:s1, 1:],
                                                cXv[:, :, 1:], Alu.is_equal)
                    # margins on md, ml
                    for m_t in (md, ml):
                        mv = m_t[:].rearrange("p (s c) -> p s c", c=C)[:, 0:ns, :]
                        for q in (0, 1, 2):
                            nc.gpsimd.tensor_copy(
                                mv[32 * q:32 * q + 32, :, 0:1],
                                mv[32 * (q + 1):32 * (q + 1) + 32, :, 1:2])
                        nc.gpsimd.memset(mv[96:128, :, 0:1], 0.0)
                    for m_t, m_dram in ((mu, mU_d), (md, mD_d), (ml, mL_d)):
                        nc.sync.dma_start(out=m_dram[:, s0 * C:s1 * C],
                                          in_=m_t[:, 0:ns * C])
                # zero-fill DRAM mask slots 260..261
                for m_dram in (mU_d, mD_d, mL_d):
                    nc.sync.dma_start(out=m_dram[:, 260 * C:262 * C], in_=zb[:, :])

                # mx DMA-in over d (all mask-phase reads of d are done)
                nc.sync.dma_start(out=d[:], in_=mx_d[:])

                # ---------------- backward ----------------
                if "b" not in phases:
                    continue
                def win_load(k):
                    w0 = k * WCH
                    nsl = min(WCH + 2, SE - w0)
                    tiles = {}
                    for name, m_dram in (("u", mU_d), ("d", mD_d), ("l", mL_d)):
                        w = win_pool.tile([128, (WCH + 2) * C], dt.bfloat16, tag="w" + name)
                        nc.sync.dma_start(out=w[:, 0:nsl * C],
                                          in_=m_dram[:, w0 * C:(w0 + nsl) * C])
                        tiles[name] = w
                    return tiles

                cur_k = (S - 1) // WCH
                wins = {cur_k: win_load(cur_k)}
                if cur_k - 1 >= 0:
                    wins[cur_k - 1] = win_load(cur_k - 1)
                for t in range(S - 1, 0, -1):
                    k = t // WCH
                    if k != cur_k:
                        cur_k = k
                        wins.pop(k + 2, None)
                        if k - 1 >= 0 and (k - 1) not in wins:
                            wins[k - 1] = win_load(k - 1)
                    W = wins[k]
                    lo = (t - k * WCH) * C
                    qlo, qhi = max(0, t - 256), min(3, t - 1)
                    P0, P1 = prange(qlo, qhi)
                    G = G_tiles[t % 2]
                    Sc = S_tiles[t % 2]
                    for q in (2, 1, 0):
                        nc.vector.tensor_copy(
                            G[32 * q:32 * q + 32, 65:66],
                            E[32 * (q + 1):32 * (q + 2), (t + 1) * C + 1:(t + 1) * C + 2])
                    nc.vector.tensor_tensor(
                        G[P0:P1, 1:65], E[P0:P1, (t + 1) * C + 1:(t + 1) * C + 65],
                        W["u"][P0:P1, lo + C + 1:lo + C + 65], Alu.mult)
                    nc.vector.tensor_tensor(
                        Sc[P0:P1, 1:65], E[P0:P1, (t + 1) * C + 2:(t + 1) * C + 66],
                        W["d"][P0:P1, lo + C + 2:lo + C + 66], Alu.mult)
                    nc.vector.tensor_tensor(G[P0:P1, 1:65], G[P0:P1, 1:65],
                                            Sc[P0:P1, 1:65], Alu.add)
                    if t == S - 1:
                        nc.vector.memset(G[96:128, 64:65], 1.0)
                    nc.vector.tensor_tensor_scan(
                        E[P0:P1, t * C:t * C + 66][:, ::-1],
                        W["l"][P0:P1, lo + 1:lo + 67][:, ::-1],
                        G[P0:P1, 0:66][:, ::-1], 0.0, Alu.mult, Alu.add)

                # ---------------- omega reduction ----------------
                if "x" not in phases:
                    continue
                nc.vector.tensor_tensor(E[:, 0:S * C], E[:, 0:S * C],
                                        d[:, 0:S * C], Alu.mult)
                nc.vector.tensor_tensor(E[:, 0:S * C], E[:, 0:S * C],
                                        d[:, 0:S * C], Alu.mult)
                nc.vector.tensor_reduce(
                    pt_t[:, 0:1],
                    E[:, 0:S * C].rearrange("p (s c) -> p s c", c=C),
                    mybir.AxisListType.XY, Alu.add)
                nc.sync.dma_start(out=out_d[:, 2 * rep:2 * rep + 1], in_=pt_t[:])

    nc.compile()
    return nc


def _make_runner(nc):
    """Cached single-device jit over the bass_exec primitive (mirrors
    bass2jax.run_bass_via_pjrt's n_cores==1 path, built once and reused)."""
    import jax
    import concourse.mybir as mybir
    from concourse import bass2jax

    bass2jax.install_neuronx_cc_hook()
    partition_name = nc.partition_id_tensor.name if nc.partition_id_tensor else None
    in_names, out_names, out_avals, zero_outs = [], [], [], []
    for alloc in nc.m.functions[0].allocations:
        if not isinstance(alloc, mybir.MemoryLocationSet):
            continue
        name = alloc.memorylocations[0].name
        if alloc.kind == "ExternalInput":
            if name != partition_name:
                in_names.append(name)
        elif alloc.kind == "ExternalOutput":
            shape = tuple(alloc.tensor_shape)
            dtype = mybir.dt.np(alloc.dtype)
            out_avals.append(jax.core.ShapedArray(shape, dtype))
            zero_outs.append(np.zeros(shape, dtype))
            out_names.append(name)
    n_params, n_outs = len(in_names), len(out_names)
    all_names = list(in_names) + list(out_names)
    if partition_name is not None:
        all_names.append(partition_name)

    def _body(*args):
        operands = list(args)
        if partition_name is not None:
            operands.append(bass2jax.partition_id_tensor())
        return tuple(bass2jax._bass_exec_p.bind(
            *operands,
            out_avals=tuple(out_avals),
            in_names=tuple(all_names),
            out_names=tuple(out_names),
            lowering_input_output_aliases=(),
            sim_require_finite=True,
            sim_require_nnan=True,
            nc=nc))

    donate = tuple(range(n_params, n_params + n_outs))
    jf = jax.jit(_body, donate_argnums=donate, keep_unused=True)
    return jf, in_names, out_names, zero_outs


def _pack_inputs(yp, yt):
    Y = np.empty((128, N), f32)
    for rep in range(NREPS):
        Y[64 * rep:64 * rep + 32] = yt[32 * rep:32 * rep + 32]
        Y[64 * rep + 32:64 * rep + 64] = yp[32 * rep:32 * rep + 32]
    return Y


def _reduce_outputs(out):
    ps = np.concatenate([out[96:128, 2 * rep + 1] for rep in range(NREPS)])
    loss_shape = float(np.mean(ps))
    pt_sum = float(sum(out[:, 2 * rep].sum() for rep in range(NREPS)))
    loss_temporal = pt_sum / B / (N * N)
    return np.array(ALPHA * loss_shape + (1.0 - ALPHA) * loss_temporal, dtype=f32)


def kernel(y_pred, y_true):
    yp = np.asarray(y_pred, dtype=f32).reshape(B, N)
    yt = np.asarray(y_true, dtype=f32).reshape(B, N)
    if "nc" not in _cache:
        _cache["nc"] = _build()
    nc = _cache["nc"]
    Y = _pack_inputs(yp, yt)
    try:
        if "jf" not in _cache:
            _cache["jf"] = _make_runner(nc)
        jf, in_names, out_names, zero_outs = _cache["jf"]
        args = [Y if nm == "Y" else _mx_const() for nm in in_names]
        zs = [np.zeros_like(z) for z in zero_outs]
        outs = jf(*args, *zs)
        out = np.asarray(outs[out_names.index("out")])
    except Exception:
        # fallback: proven-but-slower spmd driver on core 0
        from concourse.bass_utils import run_bass_kernel_spmd
        in_map = {"Y": Y}
        import concourse.mybir as mybir
        for alloc in nc.m.functions[0].allocations:
            if (isinstance(alloc, mybir.MemoryLocationSet)
                    and alloc.kind == "ExternalInput"
                    and alloc.memorylocations[0].name == "mx"):
                in_map["mx"] = np.asarray(_mx_const())
        res = run_bass_kernel_spmd(nc, [in_map], core_ids=[0])
        out = res.results[0]["out"]
    return _reduce_outputs(out)


# revision 3
# speedup vs baseline: 2.7243x; 2.7243x over previous
"""DILATE loss (soft-DTW fwd + grad, gamma=0.01 ~ hard-min) on TRN2.

Single-core, two sequential passes of 32 samples each (the DP instructions
span all 128 SBUF partitions: quadrant q holds col-block q for 32 samples on
lanes 32q+0..31). Per pass, the skewed wavefront runs 260 slots; slot t of
block q holds DP row i = t - q in a 65-float record [chain | 64 cols];
tensor_tensor_scan computes each row's min-plus recurrence in one
instruction. The soft-DTW gradient is the hard argmin-mask linear recurrence
run as a reversed scan; masks are equality-derived in batched chunks and
bounced through DRAM.

Wall-clock is dominated by the axon tunnel round trip (~85ms per
host-initiated op), so the host interface is minimized: one cached jax.jit
executable, one [128,256] f32 input (raw y values; the skewed dT/dO layouts
are built on-device by DMA), one [128,4] f32 output (pt/ps packed per pass),
and the index-difference omega matrix baked into the NEFF as a constant.
"""
import numpy as np
import ml_dtypes

bf16 = ml_dtypes.bfloat16
f32 = np.float32

ALPHA = 0.5
BIG = 1e8
B, N = 64, 256
Q, C = 4, 65
S, SE = 260, 262
NREPS = 2          # passes per core; 32 samples each
SPP = 32           # samples per pass
MCH = 8            # mask-phase chunk (slots)
WCH = 8            # backward mask window stride (slots); window covers WCH+1 slots

_cache = {}


def _mx_const():
    """[128, S*C] bf16: mx[32q+s, t, 1+l] = (i - j) for i=t-q, j=q*64+1+l,
    zeroed outside 1<=i<=N and in the chain col. Input-independent."""
    mx = np.zeros((128, S, C), f32)
    t = np.arange(S)
    for q in range(Q):
        i = t - q
        valid = (i >= 1) & (i <= N)
        j = q * 64 + np.arange(1, 65)
        X = (i[:, None] - j[None, :]).astype(f32)
        X[~valid, :] = 0.0
        mx[32 * q:32 * (q + 1), :, 1:] = X[None, :, :]
    return mx.reshape(128, S * C).astype(bf16)


def _build(nreps=NREPS, phases="dfmbx", fwdops="cms"):
    import concourse.bacc as bacc
    import concourse.tile as tile
    import concourse.mybir as mybir
    from contextlib import ExitStack

    dt = mybir.dt
    Alu = mybir.AluOpType

    nc = bacc.Bacc("TRN2", target_bir_lowering=False, debug=False)
    # rows 64r+0..31: y_true samples 32r..32r+31; rows 64r+32..63: y_pred
    Y_d = nc.dram_tensor("Y", [128, N], dt.float32, kind="ExternalInput").ap()
    out_d = nc.dram_tensor("out", [128, 2 * nreps], dt.float32,
                           kind="ExternalOutput").ap()
    mx_d = nc.inline_tensor(_mx_const(), name="mx").ap()
    mU_d = nc.dram_tensor("mU_s", [128, SE * C], dt.bfloat16).ap()
    mD_d = nc.dram_tensor("mD_s", [128, SE * C], dt.bfloat16).ap()
    mL_d = nc.dram_tensor("mL_s", [128, SE * C], dt.bfloat16).ap()

    with tile.TileContext(nc) as tc:
        with ExitStack() as ctx:
            big = ctx.enter_context(tc.tile_pool(name="big", bufs=1))
            st_pool = ctx.enter_context(tc.tile_pool(name="stage", bufs=2))
            win_pool = ctx.enter_context(tc.tile_pool(name="win", bufs=2))
            sc_pool = ctx.enter_context(tc.tile_pool(name="scr", bufs=2))

            h = big.tile([128, S * C], dt.float32, tag="h")
            d = big.tile([128, S * C], dt.bfloat16, tag="d")
            E = big.tile([128, SE * C], dt.float32, tag="E")
            dT = big.tile([128, S], dt.float32, tag="dT")
            dO = big.tile([128, 64], dt.float32, tag="dO")
            c0 = big.tile([128, C], dt.float32, tag="c0")
            c1 = big.tile([128, C], dt.float32, tag="c1")
            G0 = big.tile([128, 66], dt.float32, tag="G0")
            G1t = big.tile([128, 66], dt.float32, tag="G1t")
            S0 = big.tile([128, 66], dt.float32, tag="S0")
            S1t = big.tile([128, 66], dt.float32, tag="S1t")
            zb = big.tile([128, 2 * C], dt.bfloat16, tag="zb")
            pt_t = big.tile([128, 1], dt.float32, tag="pt_t")
            c_tiles = [c0, c1]
            G_tiles = [G0, G1t]
            S_tiles = [S0, S1t]

            for rep in range(nreps):
                # skewed dT/dO layout straight from DRAM: quadrant q gets
                # y_true shifted by q+1 cols / y_pred col-block q
                nc.vector.memset(dT[:], 0.0)
                for q in range(Q):
                    nc.sync.dma_start(
                        out=dT[32 * q:32 * q + 32, q + 1:q + 257],
                        in_=Y_d[64 * rep:64 * rep + 32, :])
                    nc.sync.dma_start(
                        out=dO[32 * q:32 * q + 32, 0:64],
                        in_=Y_d[64 * rep + 32:64 * rep + 64, 64 * q:64 * q + 64])
                # E zero on gpsimd (runs concurrent with fwd on DVE)
                nc.gpsimd.memset(E[:], 0.0)
                nc.gpsimd.memset(zb[:], 0.0)

                # D build: d[p, t*C+1+jl] = (dT[p,t]-dO[p,jl])^2  (bf16)
                if "d" not in phases:
                    continue
                nc.vector.memset(d[:], 0.0)
                DCH = 33
                for k0 in range(0, S, DCH):
                    k1 = min(k0 + DCH, S)
                    d3 = d[:].rearrange("p (s c) -> p s c", c=C)[:, k0:k1, 1:]
                    nc.vector.tensor_tensor(
                        d3, dT[:, k0:k1].unsqueeze(2).broadcast_to([128, k1 - k0, 64]),
                        dO[:].unsqueeze(1).broadcast_to([128, k1 - k0, 64]), Alu.subtract)
                    nc.vector.tensor_tensor(d3, d3, d3, Alu.mult)

                # fwd prefills
                for q in range(Q):
                    nc.vector.memset(h[32 * q:32 * q + 32, q * C:(q + 1) * C], BIG)
                nc.vector.memset(h[0:32, 0:1], 0.0)
                for ct in c_tiles:
                    nc.vector.memset(ct[0:32, 0:1], BIG)
                for gt in G_tiles:
                    nc.vector.memset(gt[:, 0:1], 0.0)
                    nc.vector.memset(gt[96:128, 65:66], 0.0)

                # ---------------- forward ----------------
                if "f" not in phases:
                    continue
                def prange(qlo, qhi):
                    P0, P1 = 32 * qlo, 32 * qhi + 32
                    cnt = P1 - P0
                    if not (cnt <= 32 or P0 == 0 or (P0 == 64 and cnt <= 64)):
                        P0 = 0
                    return P0, P1

                for t in range(1, S):
                    qlo, qhi = max(0, t - 256), min(3, t - 1)
                    P0, P1 = prange(qlo, qhi)
                    ct = c_tiles[t % 2]
                    if "c" in fwdops:
                        for q in range(max(1, qlo), qhi + 1):
                            nc.gpsimd.tensor_copy(
                                ct[32 * q:32 * q + 32, 0:1],
                                h[32 * (q - 1):32 * q, (t - 1) * C + 64:(t - 1) * C + 65])
                    if "m" in fwdops:
                        nc.vector.tensor_tensor(
                            ct[P0:P1, 1:65],
                            h[P0:P1, (t - 1) * C + 1:(t - 1) * C + 65],
                            h[P0:P1, (t - 1) * C:(t - 1) * C + 64], Alu.min)
                    if "s" in fwdops:
                        # state = min(c'_j, state) + d_j  (c' excludes d; chain in c'[0])
                        nc.vector.tensor_tensor_scan(
                            h[P0:P1, t * C:t * C + 65],
                            ct[P0:P1, 0:65],
                            d[P0:P1, t * C:t * C + 65], float(BIG), Alu.min, Alu.add)

                # loss_shape partials: V[N,M] per sample
                nc.sync.dma_start(
                    out=out_d[96:128, 2 * rep + 1:2 * rep + 2],
                    in_=h[96:128, 259 * C + 64:259 * C + 65])

                # ---------------- mask phase ----------------
                if "m" not in phases:
                    continue
                for s0 in range(1, S, MCH):
                    s1 = min(s0 + MCH, S)
                    ns = s1 - s0
                    cX = sc_pool.tile([128, MCH * C], dt.float32, tag="cX")
                    mu = st_pool.tile([128, MCH * C], dt.bfloat16, tag="mu")
                    md = st_pool.tile([128, MCH * C], dt.bfloat16, tag="md")
                    ml = st_pool.tile([128, MCH * C], dt.bfloat16, tag="ml")
                    hv = h[:].rearrange("p (s c) -> p s c", c=C)
                    dv = d[:].rearrange("p (s c) -> p s c", c=C)
                    cXv = cX[:].rearrange("p (s c) -> p s c", c=C)[:, 0:ns, :]
                    for m_t, hoff in ((mu, hv[:, s0 - 1:s1 - 1, 1:]),
                                      (md, hv[:, s0 - 1:s1 - 1, 0:64]),
                                      (ml, hv[:, s0:s1, 0:64])):
                        nc.vector.tensor_tensor(cXv[:, :, 1:], dv[:, s0:s1, 1:], hoff, Alu.add)
                        mv = m_t[:].rearrange("p (s c) -> p s c", c=C)[:, 0:ns, :]
                        nc.vector.tensor_tensor(mv[:, :, 1:], hv[:, s0:s1, 1:],
                                                cXv[:, :, 1:], Alu.is_equal)
                    # margins on md, ml
                    for m_t in (md, ml):
                        mv = m_t[:].rearrange("p (s c) -> p s c", c=C)[:, 0:ns, :]
                        for q in (0, 1, 2):
                            nc.gpsimd.tensor_copy(
                                mv[32 * q:32 * q + 32, :, 0:1],
                                mv[32 * (q + 1):32 * (q + 1) + 32, :, 1:2])
                        nc.gpsimd.memset(mv[96:128, :, 0:1], 0.0)
                    for m_t, m_dram in ((mu, mU_d), (md, mD_d), (ml, mL_d)):
                        nc.sync.dma_start(out=m_dram[:, s0 * C:s1 * C],
                                          in_=m_t[:, 0:ns * C])
                # zero-fill DRAM mask slots 260..261
                for m_dram in (mU_d, mD_d, mL_d):
                    nc.sync.dma_start(out=m_dram[:, 260 * C:262 * C], in_=zb[:, :])

                # mx DMA-in over d (all mask-phase reads of d are done)
                nc.sync.dma_start(out=d[:], in_=mx_d[:])

                # ---------------- backward ----------------
                if "b" not in phases:
                    continue
                def win_load(k):
                    w0 = k * WCH
                    nsl = min(WCH + 2, SE - w0)
                    tiles = {}
                    for name, m_dram in (("u", mU_d), ("d", mD_d), ("l", mL_d)):
                        w = win_pool.tile([128, (WCH + 2) * C], dt.bfloat16, tag="w" + name)
                        nc.sync.dma_start(out=w[:, 0:nsl * C],
                                          in_=m_dram[:, w0 * C:(w0 + nsl) * C])
                        tiles[name] = w
                    return tiles

                cur_k = (S - 1) // WCH
                wins = {cur_k: win_load(cur_k)}
                if cur_k - 1 >= 0:
                    wins[cur_k - 1] = win_load(cur_k - 1)
                for t in range(S - 1, 0, -1):
                    k = t // WCH
                    if k != cur_k:
                        cur_k = k
                        wins.pop(k + 2, None)
                        if k - 1 >= 0 and (k - 1) not in wins:
                            wins[k - 1] = win_load(k - 1)
                    W = wins[k]
                    lo = (t - k * WCH) * C
                    qlo, qhi = max(0, t - 256), min(3, t - 1)
                    P0, P1 = prange(qlo, qhi)
                    G = G_tiles[t % 2]
                    Sc = S_tiles[t % 2]
                    for q in (2, 1, 0):
                        nc.vector.tensor_copy(
                            G[32 * q:32 * q + 32, 65:66],
                            E[32 * (q + 1):32 * (q + 2), (t + 1) * C + 1:(t + 1) * C + 2])
                    nc.vector.tensor_tensor(
                        G[P0:P1, 1:65], E[P0:P1, (t + 1) * C + 1:(t + 1) * C + 65],
                        W["u"][P0:P1, lo + C + 1:lo + C + 65], Alu.mult)
                    nc.vector.tensor_tensor(
                        Sc[P0:P1, 1:65], E[P0:P1, (t + 1) * C + 2:(t + 1) * C + 66],
                        W["d"][P0:P1, lo + C + 2:lo + C + 66], Alu.mult)
                    nc.vector.tensor_tensor(G[P0:P1, 1:65], G[P0:P1, 1:65],
                                            Sc[P0:P1, 1:65], Alu.add)
                    if t == S - 1:
                        nc.vector.memset(G[96:128, 64:65], 1.0)
                    nc.vector.tensor_tensor_scan(
                        E[P0:P1, t * C:t * C + 66][:, ::-1],
                        W["l"][P0:P1, lo + 1:lo + 67][:, ::-1],
                        G[P0:P1, 0:66][:, ::-1], 0.0, Alu.mult, Alu.add)

                # ---------------- omega reduction ----------------
                if "x" not in phases:
                    continue
                nc.vector.tensor_tensor(E[:, 0:S * C], E[:, 0:S * C],
                                        d[:, 0:S * C], Alu.mult)
                nc.vector.tensor_tensor(E[:, 0:S * C], E[:, 0:S * C],
                                        d[:, 0:S * C], Alu.mult)
                nc.vector.tensor_reduce(
                    pt_t[:, 0:1],
                    E[:, 0:S * C].rearrange("p (s c) -> p s c", c=C),
                    mybir.AxisListType.XY, Alu.add)
                nc.sync.dma_start(out=out_d[:, 2 * rep:2 * rep + 1], in_=pt_t[:])

    nc.compile()
    return nc


def _make_runner(nc):
    """Cached single-device executable over the bass_exec primitive (mirrors
    bass2jax.run_bass_via_pjrt's n_cores==1 path, built once and reused).
    Prefers fast_dispatch_compile (no effect token -> C++ fast-path dispatch);
    falls back to a plain cached jax.jit."""
    import jax
    import concourse.mybir as mybir
    from concourse import bass2jax

    bass2jax.install_neuronx_cc_hook()
    partition_name = nc.partition_id_tensor.name if nc.partition_id_tensor else None
    in_names, in_zeros, out_names, out_avals, zero_outs = [], [], [], [], []
    for alloc in nc.m.functions[0].allocations:
        if not isinstance(alloc, mybir.MemoryLocationSet):
            continue
        name = alloc.memorylocations[0].name
        if alloc.kind == "ExternalInput":
            if name != partition_name:
                in_names.append(name)
                in_zeros.append(np.zeros(tuple(alloc.tensor_shape),
                                         mybir.dt.np(alloc.dtype)))
        elif alloc.kind == "ExternalOutput":
            shape = tuple(alloc.tensor_shape)
            dtype = mybir.dt.np(alloc.dtype)
            out_avals.append(jax.core.ShapedArray(shape, dtype))
            zero_outs.append(np.zeros(shape, dtype))
            out_names.append(name)
    n_params, n_outs = len(in_names), len(out_names)
    all_names = list(in_names) + list(out_names)
    if partition_name is not None:
        all_names.append(partition_name)

    def _body(*args):
        operands = list(args)
        if partition_name is not None:
            operands.append(bass2jax.partition_id_tensor())
        return tuple(bass2jax._bass_exec_p.bind(
            *operands,
            out_avals=tuple(out_avals),
            in_names=tuple(all_names),
            out_names=tuple(out_names),
            lowering_input_output_aliases=(),
            sim_require_finite=True,
            sim_require_nnan=True,
            nc=nc))

    donate = tuple(range(n_params, n_params + n_outs))
    try:
        example = list(in_zeros) + list(zero_outs)
        jf = bass2jax.fast_dispatch_compile(
            lambda: jax.jit(_body, donate_argnums=donate,
                            keep_unused=True).lower(*example).compile())
    except Exception:
        jf = jax.jit(_body, donate_argnums=donate, keep_unused=True)
    return jf, in_names, out_names, zero_outs


def _pack_inputs(yp, yt):
    Y = np.empty((128, N), f32)
    for rep in range(NREPS):
        Y[64 * rep:64 * rep + 32] = yt[32 * rep:32 * rep + 32]
        Y[64 * rep + 32:64 * rep + 64] = yp[32 * rep:32 * rep + 32]
    return Y


def _reduce_outputs(out):
    ps = np.concatenate([out[96:128, 2 * rep + 1] for rep in range(NREPS)])
    loss_shape = float(np.mean(ps))
    pt_sum = float(sum(out[:, 2 * rep].sum() for rep in range(NREPS)))
    loss_temporal = pt_sum / B / (N * N)
    return np.array(ALPHA * loss_shape + (1.0 - ALPHA) * loss_temporal, dtype=f32)


def kernel(y_pred, y_true):
    yp = np.asarray(y_pred, dtype=f32).reshape(B, N)
    yt = np.asarray(y_true, dtype=f32).reshape(B, N)
    if "nc" not in _cache:
        _cache["nc"] = _build()
    nc = _cache["nc"]
    Y = _pack_inputs(yp, yt)
    try:
        if "jf" not in _cache:
            _cache["jf"] = _make_runner(nc)
        jf, in_names, out_names, zero_outs = _cache["jf"]
        args = [Y if nm == "Y" else _mx_const() for nm in in_names]
        zs = [np.zeros_like(z) for z in zero_outs]
        outs = jf(*args, *zs)
        out = np.asarray(outs[out_names.index("out")])
    except Exception:
        # fallback: proven-but-slower spmd driver on core 0
        from concourse.bass_utils import run_bass_kernel_spmd
        in_map = {"Y": Y}
        import concourse.mybir as mybir
        for alloc in nc.m.functions[0].allocations:
            if (isinstance(alloc, mybir.MemoryLocationSet)
                    and alloc.kind == "ExternalInput"
                    and alloc.memorylocations[0].name == "mx"):
                in_map["mx"] = np.asarray(_mx_const())
        res = run_bass_kernel_spmd(nc, [in_map], core_ids=[0])
        out = res.results[0]["out"]
    return _reduce_outputs(out)


# revision 8
# speedup vs baseline: 2.9509x; 1.0832x over previous
"""DILATE loss (soft-DTW fwd + grad, gamma=0.01 ~ hard-min) on TRN2.

Single-core, two sequential passes of 32 samples each (the DP instructions
span all 128 SBUF partitions: quadrant q holds col-block q for 32 samples on
lanes 32q+0..31). Per pass, the skewed wavefront runs 260 slots; slot t of
block q holds DP row i = t - q in a 65-float record [chain | 64 cols];
tensor_tensor_scan computes each row's min-plus recurrence in one
instruction. The soft-DTW gradient is the hard argmin-mask linear recurrence
run as a reversed scan; masks are equality-derived in batched chunks and
bounced through DRAM.

Wall-clock is dominated by the axon tunnel round trip (~85ms per
host-initiated op), so the host interface is minimized: one cached jax.jit
executable, one [128,256] f32 input (raw y values; the skewed dT/dO layouts
are built on-device by DMA), one [128,4] f32 output (pt/ps packed per pass),
and the index-difference omega matrix baked into the NEFF as a constant.
"""
import numpy as np
import ml_dtypes

bf16 = ml_dtypes.bfloat16
f32 = np.float32

ALPHA = 0.5
BIG = 1e8
B, N = 64, 256
Q, C = 4, 65
S, SE = 260, 262
NREPS = 2          # passes per core; 32 samples each
SPP = 32           # samples per pass
MCH = 8            # mask-phase chunk (slots)
WCH = 8            # backward mask window stride (slots); window covers WCH+1 slots

_cache = {}


def _mx_const():
    """[128, S*C] bf16: mx[32q+s, t, 1+l] = (i - j) for i=t-q, j=q*64+1+l,
    zeroed outside 1<=i<=N and in the chain col. Input-independent."""
    mx = np.zeros((128, S, C), f32)
    t = np.arange(S)
    for q in range(Q):
        i = t - q
        valid = (i >= 1) & (i <= N)
        j = q * 64 + np.arange(1, 65)
        X = (i[:, None] - j[None, :]).astype(f32)
        X[~valid, :] = 0.0
        mx[32 * q:32 * (q + 1), :, 1:] = X[None, :, :]
    return mx.reshape(128, S * C).astype(bf16)


def _build(nreps=NREPS, phases="dfmbx", fwdops="cms", eng="F"):
    # eng flags: F fwd chain copies on DVE (no cross-engine sync in serial loop)
    #            B bwd G copies on gpsimd   S bwd G copies on scalar engine
    #            M mask cX adds on gpsimd   D D-build products on gpsimd
    import concourse.bacc as bacc
    import concourse.tile as tile
    import concourse.mybir as mybir
    from contextlib import ExitStack

    dt = mybir.dt
    Alu = mybir.AluOpType

    nc = bacc.Bacc("TRN2", target_bir_lowering=False, debug=False)
    # rows 64r+0..31: y_true samples 32r..32r+31; rows 64r+32..63: y_pred
    Y_d = nc.dram_tensor("Y", [128, N], dt.float32, kind="ExternalInput").ap()
    out_d = nc.dram_tensor("out", [128, 2 * nreps], dt.float32,
                           kind="ExternalOutput").ap()
    mx_d = nc.inline_tensor(_mx_const(), name="mx").ap()
    mU_d = nc.dram_tensor("mU_s", [128, SE * C], dt.bfloat16).ap()
    mD_d = nc.dram_tensor("mD_s", [128, SE * C], dt.bfloat16).ap()
    mL_d = nc.dram_tensor("mL_s", [128, SE * C], dt.bfloat16).ap()

    with tile.TileContext(nc) as tc:
        with ExitStack() as ctx:
            big = ctx.enter_context(tc.tile_pool(name="big", bufs=1))
            st_pool = ctx.enter_context(tc.tile_pool(name="stage", bufs=2))
            win_pool = ctx.enter_context(tc.tile_pool(name="win", bufs=2))
            sc_pool = ctx.enter_context(tc.tile_pool(name="scr", bufs=2))

            h = big.tile([128, S * C], dt.float32, tag="h")
            d = big.tile([128, S * C], dt.bfloat16, tag="d")
            E = big.tile([128, SE * C], dt.float32, tag="E")
            dT = big.tile([128, S], dt.float32, tag="dT")
            dO = big.tile([128, 64], dt.float32, tag="dO")
            c0 = big.tile([128, C], dt.float32, tag="c0")
            c1 = big.tile([128, C], dt.float32, tag="c1")
            G0 = big.tile([128, 66], dt.float32, tag="G0")
            G1t = big.tile([128, 66], dt.float32, tag="G1t")
            S0 = big.tile([128, 66], dt.float32, tag="S0")
            S1t = big.tile([128, 66], dt.float32, tag="S1t")
            zb = big.tile([128, 2 * C], dt.bfloat16, tag="zb")
            pt_t = big.tile([128, 1], dt.float32, tag="pt_t")
            c_tiles = [c0, c1]
            G_tiles = [G0, G1t]
            S_tiles = [S0, S1t]

            for rep in range(nreps):
                # skewed dT/dO layout straight from DRAM: quadrant q gets
                # y_true shifted by q+1 cols / y_pred col-block q
                nc.vector.memset(dT[:], 0.0)
                for q in range(Q):
                    nc.sync.dma_start(
                        out=dT[32 * q:32 * q + 32, q + 1:q + 257],
                        in_=Y_d[64 * rep:64 * rep + 32, :])
                    nc.sync.dma_start(
                        out=dO[32 * q:32 * q + 32, 0:64],
                        in_=Y_d[64 * rep + 32:64 * rep + 64, 64 * q:64 * q + 64])
                # E zero on gpsimd (runs concurrent with fwd on DVE)
                nc.gpsimd.memset(E[:], 0.0)
                nc.gpsimd.memset(zb[:], 0.0)

                # D build: d[p, t*C+1+jl] = (dT[p,t]-dO[p,jl])^2  (bf16)
                if "d" not in phases:
                    continue
                nc.vector.memset(d[:], 0.0)
                DCH = 33
                for k0 in range(0, S, DCH):
                    k1 = min(k0 + DCH, S)
                    d3 = d[:].rearrange("p (s c) -> p s c", c=C)[:, k0:k1, 1:]
                    deng = nc.gpsimd if "D" in eng else nc.vector
                    deng.tensor_tensor(
                        d3, dT[:, k0:k1].unsqueeze(2).broadcast_to([128, k1 - k0, 64]),
                        dO[:].unsqueeze(1).broadcast_to([128, k1 - k0, 64]), Alu.subtract)
                    deng.tensor_tensor(d3, d3, d3, Alu.mult)

                # fwd prefills
                for q in range(Q):
                    nc.vector.memset(h[32 * q:32 * q + 32, q * C:(q + 1) * C], BIG)
                nc.vector.memset(h[0:32, 0:1], 0.0)
                for ct in c_tiles:
                    nc.vector.memset(ct[0:32, 0:1], BIG)
                for gt in G_tiles:
                    nc.vector.memset(gt[:, 0:1], 0.0)
                    nc.vector.memset(gt[96:128, 65:66], 0.0)

                # ---------------- forward ----------------
                if "f" not in phases:
                    continue
                def prange(qlo, qhi):
                    P0, P1 = 32 * qlo, 32 * qhi + 32
                    cnt = P1 - P0
                    if not (cnt <= 32 or P0 == 0 or (P0 == 64 and cnt <= 64)):
                        P0 = 0
                    return P0, P1

                for t in range(1, S):
                    qlo, qhi = max(0, t - 256), min(3, t - 1)
                    P0, P1 = prange(qlo, qhi)
                    ct = c_tiles[t % 2]
                    if "c" in fwdops:
                        ceng = nc.vector if "F" in eng else nc.gpsimd
                        cq0, cq1 = max(1, qlo), qhi
                        if "2" in eng and cq1 >= cq0:
                            # dst base 32 is illegal >32 rows; split at q=2 (base 64)
                            if cq0 == 1:
                                ceng.tensor_copy(
                                    ct[32:64, 0:1],
                                    h[0:32, (t - 1) * C + 64:(t - 1) * C + 65])
                                cq0 = 2
                            if cq1 >= cq0:
                                ceng.tensor_copy(
                                    ct[32 * cq0:32 * cq1 + 32, 0:1],
                                    h[32 * (cq0 - 1):32 * cq1,
                                      (t - 1) * C + 64:(t - 1) * C + 65])
                        else:
                            for q in range(cq0, cq1 + 1):
                                ceng.tensor_copy(
                                    ct[32 * q:32 * q + 32, 0:1],
                                    h[32 * (q - 1):32 * q, (t - 1) * C + 64:(t - 1) * C + 65])
                    if "m" in fwdops:
                        nc.vector.tensor_tensor(
                            ct[P0:P1, 1:65],
                            h[P0:P1, (t - 1) * C + 1:(t - 1) * C + 65],
                            h[P0:P1, (t - 1) * C:(t - 1) * C + 64], Alu.min)
                    if "s" in fwdops:
                        # state = min(c'_j, state) + d_j  (c' excludes d; chain in c'[0])
                        nc.vector.tensor_tensor_scan(
                            h[P0:P1, t * C:t * C + 65],
                            ct[P0:P1, 0:65],
                            d[P0:P1, t * C:t * C + 65], float(BIG), Alu.min, Alu.add)

                # loss_shape partials: V[N,M] per sample
                nc.sync.dma_start(
                    out=out_d[96:128, 2 * rep + 1:2 * rep + 2],
                    in_=h[96:128, 259 * C + 64:259 * C + 65])

                # ---------------- mask phase ----------------
                if "m" not in phases:
                    continue
                for s0 in range(1, S, MCH):
                    s1 = min(s0 + MCH, S)
                    ns = s1 - s0
                    cX = sc_pool.tile([128, MCH * C], dt.float32, tag="cX")
                    mu = st_pool.tile([128, MCH * C], dt.bfloat16, tag="mu")
                    md = st_pool.tile([128, MCH * C], dt.bfloat16, tag="md")
                    ml = st_pool.tile([128, MCH * C], dt.bfloat16, tag="ml")
                    hv = h[:].rearrange("p (s c) -> p s c", c=C)
                    dv = d[:].rearrange("p (s c) -> p s c", c=C)
                    cXv = cX[:].rearrange("p (s c) -> p s c", c=C)[:, 0:ns, :]
                    for m_t, hoff in ((mu, hv[:, s0 - 1:s1 - 1, 1:]),
                                      (md, hv[:, s0 - 1:s1 - 1, 0:64]),
                                      (ml, hv[:, s0:s1, 0:64])):
                        meng = nc.gpsimd if "M" in eng else nc.vector
                        meng.tensor_tensor(cXv[:, :, 1:], dv[:, s0:s1, 1:], hoff, Alu.add)
                        mv = m_t[:].rearrange("p (s c) -> p s c", c=C)[:, 0:ns, :]
                        nc.vector.tensor_tensor(mv[:, :, 1:], hv[:, s0:s1, 1:],
                                                cXv[:, :, 1:], Alu.is_equal)
                    # margins on md, ml
                    for m_t in (md, ml):
                        mv = m_t[:].rearrange("p (s c) -> p s c", c=C)[:, 0:ns, :]
                        for q in (0, 1, 2):
                            nc.gpsimd.tensor_copy(
                                mv[32 * q:32 * q + 32, :, 0:1],
                                mv[32 * (q + 1):32 * (q + 1) + 32, :, 1:2])
                        nc.gpsimd.memset(mv[96:128, :, 0:1], 0.0)
                    for m_t, m_dram in ((mu, mU_d), (md, mD_d), (ml, mL_d)):
                        nc.sync.dma_start(out=m_dram[:, s0 * C:s1 * C],
                                          in_=m_t[:, 0:ns * C])
                # zero-fill DRAM mask slots 260..261
                for m_dram in (mU_d, mD_d, mL_d):
                    nc.sync.dma_start(out=m_dram[:, 260 * C:262 * C], in_=zb[:, :])

                # mx DMA-in over d (all mask-phase reads of d are done)
                nc.sync.dma_start(out=d[:], in_=mx_d[:])

                # ---------------- backward ----------------
                if "b" not in phases:
                    continue
                def win_load(k):
                    w0 = k * WCH
                    nsl = min(WCH + 2, SE - w0)
                    tiles = {}
                    for name, m_dram in (("u", mU_d), ("d", mD_d), ("l", mL_d)):
                        w = win_pool.tile([128, (WCH + 2) * C], dt.bfloat16, tag="w" + name)
                        nc.sync.dma_start(out=w[:, 0:nsl * C],
                                          in_=m_dram[:, w0 * C:(w0 + nsl) * C])
                        tiles[name] = w
                    return tiles

                cur_k = (S - 1) // WCH
                wins = {cur_k: win_load(cur_k)}
                if cur_k - 1 >= 0:
                    wins[cur_k - 1] = win_load(cur_k - 1)
                for t in range(S - 1, 0, -1):
                    k = t // WCH
                    if k != cur_k:
                        cur_k = k
                        wins.pop(k + 2, None)
                        if k - 1 >= 0 and (k - 1) not in wins:
                            wins[k - 1] = win_load(k - 1)
                    W = wins[k]
                    lo = (t - k * WCH) * C
                    qlo, qhi = max(0, t - 256), min(3, t - 1)
                    P0, P1 = prange(qlo, qhi)
                    G = G_tiles[t % 2]
                    Sc = S_tiles[t % 2]
                    beng = nc.gpsimd if "B" in eng else nc.vector
                    bcopy = (nc.scalar.copy if "S" in eng else beng.tensor_copy)
                    if "2" in eng:
                        bcopy(G[0:96, 65:66],
                              E[32:128, (t + 1) * C + 1:(t + 1) * C + 2])
                    else:
                        for q in (2, 1, 0):
                            bcopy(
                                G[32 * q:32 * q + 32, 65:66],
                                E[32 * (q + 1):32 * (q + 2), (t + 1) * C + 1:(t + 1) * C + 2])
                    nc.vector.tensor_tensor(
                        G[P0:P1, 1:65], E[P0:P1, (t + 1) * C + 1:(t + 1) * C + 65],
                        W["u"][P0:P1, lo + C + 1:lo + C + 65], Alu.mult)
                    nc.vector.tensor_tensor(
                        Sc[P0:P1, 1:65], E[P0:P1, (t + 1) * C + 2:(t + 1) * C + 66],
                        W["d"][P0:P1, lo + C + 2:lo + C + 66], Alu.mult)
                    nc.vector.tensor_tensor(G[P0:P1, 1:65], G[P0:P1, 1:65],
                                            Sc[P0:P1, 1:65], Alu.add)
                    if t == S - 1:
                        nc.vector.memset(G[96:128, 64:65], 1.0)
                    nc.vector.tensor_tensor_scan(
                        E[P0:P1, t * C:t * C + 66][:, ::-1],
                        W["l"][P0:P1, lo + 1:lo + 67][:, ::-1],
                        G[P0:P1, 0:66][:, ::-1], 0.0, Alu.mult, Alu.add)

                # ---------------- omega reduction ----------------
                if "x" not in phases:
                    continue
                nc.vector.tensor_tensor(E[:, 0:S * C], E[:, 0:S * C],
                                        d[:, 0:S * C], Alu.mult)
                nc.vector.tensor_tensor(E[:, 0:S * C], E[:, 0:S * C],
                                        d[:, 0:S * C], Alu.mult)
                nc.vector.tensor_reduce(
                    pt_t[:, 0:1],
                    E[:, 0:S * C].rearrange("p (s c) -> p s c", c=C),
                    mybir.AxisListType.XY, Alu.add)
                nc.sync.dma_start(out=out_d[:, 2 * rep:2 * rep + 1], in_=pt_t[:])

    nc.compile()
    return nc


def _make_runner(nc):
    """Cached single-device executable over the bass_exec primitive (mirrors
    bass2jax.run_bass_via_pjrt's n_cores==1 path, built once and reused).
    Prefers fast_dispatch_compile (no effect token -> C++ fast-path dispatch);
    falls back to a plain cached jax.jit."""
    import jax
    import concourse.mybir as mybir
    from concourse import bass2jax

    bass2jax.install_neuronx_cc_hook()
    partition_name = nc.partition_id_tensor.name if nc.partition_id_tensor else None
    in_names, in_zeros, out_names, out_avals, zero_outs = [], [], [], [], []
    for alloc in nc.m.functions[0].allocations:
        if not isinstance(alloc, mybir.MemoryLocationSet):
            continue
        name = alloc.memorylocations[0].name
        if alloc.kind == "ExternalInput":
            if name != partition_name:
                in_names.append(name)
                in_zeros.append(np.zeros(tuple(alloc.tensor_shape),
                                         mybir.dt.np(alloc.dtype)))
        elif alloc.kind == "ExternalOutput":
            shape = tuple(alloc.tensor_shape)
            dtype = mybir.dt.np(alloc.dtype)
            out_avals.append(jax.core.ShapedArray(shape, dtype))
            zero_outs.append(np.zeros(shape, dtype))
            out_names.append(name)
    n_params, n_outs = len(in_names), len(out_names)
    all_names = list(in_names) + list(out_names)
    if partition_name is not None:
        all_names.append(partition_name)

    def _body(*args):
        operands = list(args)
        if partition_name is not None:
            operands.append(bass2jax.partition_id_tensor())
        return tuple(bass2jax._bass_exec_p.bind(
            *operands,
            out_avals=tuple(out_avals),
            in_names=tuple(all_names),
            out_names=tuple(out_names),
            lowering_input_output_aliases=(),
            sim_require_finite=True,
            sim_require_nnan=True,
            nc=nc))

    donate = tuple(range(n_params, n_params + n_outs))
    try:
        example = list(in_zeros) + list(zero_outs)
        jf = bass2jax.fast_dispatch_compile(
            lambda: jax.jit(_body, donate_argnums=donate,
                            keep_unused=True).lower(*example).compile())
    except Exception:
        jf = jax.jit(_body, donate_argnums=donate, keep_unused=True)
    return jf, in_names, out_names, zero_outs


def _pack_inputs(yp, yt):
    Y = np.empty((128, N), f32)
    for rep in range(NREPS):
        Y[64 * rep:64 * rep + 32] = yt[32 * rep:32 * rep + 32]
        Y[64 * rep + 32:64 * rep + 64] = yp[32 * rep:32 * rep + 32]
    return Y


def _reduce_outputs(out):
    ps = np.concatenate([out[96:128, 2 * rep + 1] for rep in range(NREPS)])
    loss_shape = float(np.mean(ps))
    pt_sum = float(sum(out[:, 2 * rep].sum() for rep in range(NREPS)))
    loss_temporal = pt_sum / B / (N * N)
    return np.array(ALPHA * loss_shape + (1.0 - ALPHA) * loss_temporal, dtype=f32)


def kernel(y_pred, y_true):
    yp = np.asarray(y_pred, dtype=f32).reshape(B, N)
    yt = np.asarray(y_true, dtype=f32).reshape(B, N)
    if "nc" not in _cache:
        _cache["nc"] = _build()
    nc = _cache["nc"]
    Y = _pack_inputs(yp, yt)
    try:
        if "jf" not in _cache:
            _cache["jf"] = _make_runner(nc)
        jf, in_names, out_names, zero_outs = _cache["jf"]
        args = [Y if nm == "Y" else _mx_const() for nm in in_names]
        zs = [np.zeros_like(z) for z in zero_outs]
        outs = jf(*args, *zs)
        out = np.asarray(outs[out_names.index("out")])
    except Exception:
        # fallback: proven-but-slower spmd driver on core 0
        from concourse.bass_utils import run_bass_kernel_spmd
        in_map = {"Y": Y}
        import concourse.mybir as mybir
        for alloc in nc.m.functions[0].allocations:
            if (isinstance(alloc, mybir.MemoryLocationSet)
                    and alloc.kind == "ExternalInput"
                    and alloc.memorylocations[0].name == "mx"):
                in_map["mx"] = np.asarray(_mx_const())
        res = run_bass_kernel_spmd(nc, [in_map], core_ids=[0])
        out = res.results[0]["out"]
    return _reduce_outputs(out)
